# revision 16
# baseline (speedup 1.0000x reference)
"""Bahdanau-attention kernel for Trainium2 (8 NeuronCores, data-parallel over batch).

Computes, for each batch b:
    q[b]    = v * (W_w @ prev[b] + W_b + U_b)            (host, tiny)
    U'      = v[:, None] * U_w                            (host, tiny)
    e[b,t]  = sum_h relu(q[b,h] + (U' @ enc[b,t])_h)      (device)
    alpha   = softmax(e[b, :])                            (device)
    out[b]  = sum_t alpha[t] * enc[b,t,:]                 (device)

The v>0 fold is exact: v_h * relu(x_h) == relu(v_h * x_h) for v_h >= 0.

Device strategy (per core: 4 batches, enc slice [4, 4096, 1024] fp32 = 64 MB
streamed from HBM exactly once, cast fp32->fp16 during the DMA — fp16's
10-bit mantissa matches the tf32-grade rounding f32r gives on HW, at half
the byte width, 1 cyc/row PE transposes, and FWL fast weight loads):
  - enc tiles [t=128, c=1024] fp16 stay SBUF-resident for the batch.
  - PE transposes each tile chunk-wise to [c, t] (fp16, PSUM), DVE copies
    the result to SBUF.
  - U-matmul in fp16 accumulates [t=128, h=256] in fp32 PSUM on top of a
    ones-row x q bias matmul.
  - ACT fused relu+row-reduce produces the energy column per tile.
  - Exact fp32 two-level softmax: per-partition max shift via the ACT bias,
    then a one-partition fixup; cross-partition gather/scatter rides PE
    transposes / a K=1 matmul.
  - Pass-2 weighted sum: alpha column as stationary, natural enc tile as
    moving operand, accumulated into PSUM [1, 1024].

Toolchain notes: the module is built as a Bacc (not raw Bass) so multi-wait
instructions get legalized into event semaphores and the walrus single-wait
LDWEIGHTS limit is respected. Matmul inputs must not mix 16/32-bit dtypes;
the softmax's per-partition max is rounded to fp16 FIRST and the rounded
value used in both exponents so z'*g composes exactly.
"""

import sys

import numpy as np

sys.path.insert(0, "/opt/trn_rl_repo")

import concourse.bacc as bacc
import concourse.mybir as mybir
import concourse.tile as tile
from concourse.bass import ts
from concourse.bass_utils import run_bass_kernel_spmd
from concourse.masks import make_identity

B, T, C, H, D = 32, 4096, 1024, 256, 512
NCORES = 8
BPC = B // NCORES  # batches per core

F32 = mybir.dt.float32
F32R = mybir.dt.float32r
F16 = mybir.dt.float16
BF16 = mybir.dt.bfloat16

P = 128            # partitions / t-tile size
CK = C // P        # 8 c-chunks per tile
NT = T // P        # 32 t-tiles per batch


def build_bass(bpc: int = BPC, n_tiles: int = NT, repeat: int = 1):
    nc = bacc.Bacc(target_bir_lowering=False, trn_type="TRN2")

    enc = nc.dram_tensor("enc", [bpc, n_tiles * P, C], F32, kind="ExternalInput")
    # q rows packed on one partition: [1, bpc*H]
    qrow = nc.dram_tensor("qrow", [1, bpc * H], F32, kind="ExternalInput")
    # U' transposed, pre-arranged host-side as [p, chunk, h] with c = chunk*128 + p
    ut = nc.dram_tensor("ut", [P, CK, H], F32, kind="ExternalInput")
    out = nc.dram_tensor("out", [bpc, C], F32, kind="ExternalOutput")

    enc_ap = enc.ap()
    out_ap = out.ap()

    with tile.TileContext(nc) as tc:
        TG = 4  # t-tiles per DMA (2 MB transfers amortize SWDGE issue cost)
        with (
            tc.tile_pool(name="singles", bufs=1) as singles,
            tc.tile_pool(name="enc_pool", bufs=2 * (n_tiles // TG)) as enc_pool,
            tc.tile_pool(name="encT_pool", bufs=3) as encT_pool,
            tc.tile_pool(name="relu_pool", bufs=3) as relu_pool,
            tc.tile_pool(name="batch_pool", bufs=3) as batch_pool,
            tc.tile_pool(name="small_pool", bufs=2) as small_pool,
            tc.tile_pool(name="outst_pool", bufs=2) as outst_pool,
            tc.tile_pool(name="ps_tp", bufs=2, space="PSUM") as ps_tp,
            tc.tile_pool(name="ps_sm", bufs=1, space="PSUM") as ps_sm,
            tc.tile_pool(name="ps_um", bufs=1, space="PSUM") as ps_um,
            tc.tile_pool(name="ps_c", bufs=1, space="PSUM") as ps_c,
        ):
            # --- constants, all funneled through DVE so PE sees one clock ---
            ident_stage = singles.tile([P, P], F32)
            make_identity(nc, ident_stage)
            ut_stage = singles.tile([P, CK, H], F32)
            nc.gpsimd.dma_start(out=ut_stage, in_=ut.ap())
            q_stage = singles.tile([1, bpc * H], F32)
            nc.gpsimd.dma_start(out=q_stage, in_=qrow.ap())

            ones_row_f = singles.tile([1, P], F32)
            nc.vector.memset(ones_row_f, 1.0)
            ones_row = singles.tile([1, P], F16)
            nc.vector.tensor_copy(ones_row, ones_row_f)
            q_s = singles.tile([1, bpc * H], F16)
            nc.vector.tensor_copy(q_s, q_stage)
            ut_s = singles.tile([P, CK, H], F16)
            nc.vector.tensor_copy(ut_s, ut_stage)
            ident_h = singles.tile([P, P], F16)
            nc.vector.tensor_copy(ident_h, ident_stage)
            # zeros operand: lets the PSUM->SBUF evacuation run as
            # tensor_tensor(+0), which only uses DVE's dedicated 1-port mode.
            # A plain tensor_copy enters 2-port perf mode and locks GpSimd out
            # of the shared SBUF port pair, starving SWDGE descriptor
            # generation for the enc cast-DMAs (documented ~5x DMA stall).
            zeros_c2 = singles.tile([P, 2 * C], F16)
            nc.vector.memset(zeros_c2, 0.0)

            def pass1(b):
                enc_tiles = []
                e_buf = batch_pool.tile([P, n_tiles], F32, tag="ebuf")
                for jg in range(n_tiles // TG):
                    enc_g = enc_pool.tile([P, TG, C], F16, tag="enc")
                    src_ap = enc_ap[b, ts(jg, TG * P), :].rearrange(
                        "(a p) c -> p a c", p=P
                    )
                    nc.gpsimd.dma_start(out=enc_g, in_=src_ap)
                    enc_tiles.extend(enc_g[:, tt, :] for tt in range(TG))
                for jp in range(n_tiles // 2):
                    # super-tile: transpose TWO t-tiles into one [P, 2, C] f16
                    # PSUM pair (2 banks) and evacuate with ONE DVE
                    # tensor_add — halves the PE<->DVE semaphore traffic.
                    encT2 = encT_pool.tile([P, 2, C], F16, tag="encT")
                    tp2 = ps_tp.tile([P, 2, C], F16, tag="tp")
                    for tt in range(2):
                        enc_t = enc_tiles[2 * jp + tt]
                        for k in range(CK):
                            nc.tensor.transpose(
                                tp2[:, tt, ts(k, P)], enc_t[:, ts(k, P)], ident_h
                            )
                    nc.vector.tensor_add(
                        encT2.rearrange("p a c -> p (a c)"),
                        tp2.rearrange("p a c -> p (a c)"),
                        zeros_c2,
                    )
                    for tt in range(2):
                        j = 2 * jp + tt
                        # U-matmul: psum[t,h] = q[h] + sum_c encT[c,t]^T ut[c,h]
                        um = ps_um.tile([P, H], F32, tag="um")
                        nc.tensor.matmul(
                            um,
                            ones_row,
                            q_s[:, b * H : (b + 1) * H],
                            start=True,
                            stop=False,
                        )
                        for k in range(CK):
                            nc.tensor.matmul(
                                um,
                                encT2[:, tt, ts(k, P)],
                                ut_s[:, k, :],
                                start=False,
                                stop=(k == CK - 1),
                            )

                        # e[t] = sum_h relu(um[t, h])  (ACT, fused reduce)
                        relu_sc = relu_pool.tile([P, H], BF16, tag="relu")
                        nc.scalar.activation(
                            out=relu_sc,
                            in_=um,
                            func=mybir.ActivationFunctionType.Relu,
                            accum_out=e_buf[:, j : j + 1],
                        )
                return enc_tiles, e_buf

            def tail(b, enc_tiles, e_buf):
                # ------- softmax (exact fp32, two-level, PE transposes) -------
                # z'[p,j] = exp(e[p,j] - mp[p]) with the per-partition max mp
                # (ACT bias is per-partition, so no broadcast needed), then a
                # one-partition fixup computes g[p] = exp(mp[p]-M)/S and
                # alpha = z' * g  ==  exp(e-M)/S exactly. Cross-partition
                # gather/scatter rides the PE transpose (sub-us) instead of
                # SBUF->SBUF DMA (~1.5us fixed each).
                ms = small_pool.tile([P, 2], F32, tag="ms")
                nc.vector.tensor_reduce(
                    ms[:, 0:1], e_buf, axis=mybir.AxisListType.X,
                    op=mybir.AluOpType.max,
                )
                ms_r = small_pool.tile([P, 2], F16, tag="ms_r")
                nc.vector.tensor_copy(ms_r[:, 0:1], ms[:, 0:1])
                mpneg = small_pool.tile([P, 1], F32, tag="mpneg")
                nc.vector.tensor_scalar_mul(mpneg, ms_r[:, 0:1], -1.0)
                z = batch_pool.tile([P, n_tiles], F32, tag="z")
                nc.scalar.activation(
                    out=z,
                    in_=e_buf,
                    func=mybir.ActivationFunctionType.Exp,
                    bias=mpneg,
                    accum_out=ms[:, 1:2],
                )
                # gather each column onto partition 0 via PE transposes
                # (f32r rounding copies keep the BIR verifier happy)
                nc.vector.tensor_copy(ms_r[:, 1:2], ms[:, 1:2])
                sm_rows = ps_sm.tile([1, 2, P], F16, tag="smr")
                mrow_ps = sm_rows[:, 0, :]
                nc.tensor.transpose(mrow_ps, ms_r[:, 0:1], ident_h)
                srow_ps = sm_rows[:, 1, :]
                nc.tensor.transpose(srow_ps, ms_r[:, 1:2], ident_h)
                mrow = small_pool.tile([1, P], F32, tag="mrow")
                nc.vector.tensor_copy(mrow, mrow_ps)
                srow = small_pool.tile([1, P], F32, tag="srow")
                nc.vector.tensor_copy(srow, srow_ps)
                mtot = small_pool.tile([1, 1], F32, tag="mtot")
                nc.vector.tensor_reduce(
                    mtot, mrow, axis=mybir.AxisListType.X, op=mybir.AluOpType.max
                )
                mtneg = small_pool.tile([1, 1], F32, tag="mtneg")
                nc.vector.tensor_scalar_mul(mtneg, mtot, -1.0)
                grow = small_pool.tile([1, P], F32, tag="grow")
                nc.scalar.activation(
                    out=grow,
                    in_=mrow,
                    func=mybir.ActivationFunctionType.Exp,
                    bias=mtneg,
                )
                wrow = small_pool.tile([1, P], F32, tag="wrow")
                nc.vector.tensor_mul(wrow, grow, srow)
                stot = small_pool.tile([1, 1], F32, tag="stot")
                nc.vector.tensor_reduce(
                    stot, wrow, axis=mybir.AxisListType.X, op=mybir.AluOpType.add
                )
                rec = small_pool.tile([1, 1], F32, tag="rec")
                nc.vector.reciprocal(rec, stot)
                gsrow = small_pool.tile([1, P], F32, tag="gsrow")
                nc.vector.tensor_scalar_mul(gsrow, grow, rec)
                gsrow_r = small_pool.tile([1, P], F16, tag="gsrow_r")
                nc.vector.tensor_copy(gsrow_r, gsrow)
                # scatter g[p]/S back to one element per partition via a
                # K=1 matmul: out[p, 0] = gsrow[p] * 1
                gscol_full = ps_um.tile([P, H], F32, tag="um")
                gscol_ps = gscol_full[:, 0:32]
                nc.tensor.matmul(
                    gscol_ps, gsrow_r, ones_row[:, 0:32], start=True, stop=True
                )
                gscol = small_pool.tile([P, 1], F32, tag="gscol")
                nc.vector.tensor_copy(gscol, gscol_ps[:, 0:1])
                alpha = batch_pool.tile([P, n_tiles], F16, tag="alpha")
                nc.vector.tensor_scalar_mul(alpha, z, gscol)

                # ---------------- pass 2: weighted sum ----------------
                cps = ps_c.tile([1, 2, D], F32, tag="cps")
                for j in range(n_tiles):
                    for h in range(2):
                        nc.tensor.matmul(
                            cps[:, h, :],
                            alpha[:, j : j + 1],
                            enc_tiles[j][:, ts(h, D)],
                            start=(j == 0),
                            stop=(j == n_tiles - 1),
                        )
                c_st = outst_pool.tile([1, C], F32, tag="cst")
                nc.vector.tensor_copy(c_st, cps.rearrange("p a b -> p (a b)"))
                nc.sync.dma_start(out=out_ap[b : b + 1, :], in_=c_st)

            def batches():
                # software-pipelined emission: batch b's softmax+pass-2 is
                # emitted AFTER batch b+1's pass-1, so the scheduler keeps the
                # PE fed with b+1 transposes/U-matmuls while b's softmax
                # dependency chain runs on DVE/ACT. enc_pool holds exactly two
                # batches (16 groups).
                prev = None
                for b in range(bpc):
                    cur = (b, *pass1(b))
                    if prev is not None:
                        tail(*prev)
                    prev = cur
                tail(*prev)

            if repeat == 1:
                batches()
            else:
                with tc.For_i(0, repeat, 1):
                    batches()

    return nc


_NC_CACHE: dict = {}


def _get_nc(bpc=BPC, n_tiles=NT):
    key = (bpc, n_tiles)
    if key not in _NC_CACHE:
        nc = build_bass(bpc, n_tiles)
        if not nc.is_finalized():
            nc.finalize()
        _NC_CACHE[key] = nc
    return _NC_CACHE[key]


def _host_prep(previous_decoder_hidden_state, W_w, W_b, U_w, U_b, v):
    prev = np.asarray(previous_decoder_hidden_state, dtype=np.float32)[:, 0, :]
    W_w = np.asarray(W_w, dtype=np.float32)
    U_w = np.asarray(U_w, dtype=np.float32)
    v = np.asarray(v, dtype=np.float32)
    bias = np.asarray(W_b, dtype=np.float32) + np.asarray(U_b, dtype=np.float32)
    q_all = (v[None, :] * (prev @ W_w.T + bias)).astype(np.float32)  # [B, H]
    up = (v[:, None] * U_w).astype(np.float32)  # [H, C]
    # ut_host[p, k, h] = up.T[k*128 + p, h]
    ut_host = np.ascontiguousarray(up.T.reshape(CK, P, H).transpose(1, 0, 2))
    return q_all, ut_host


def kernel(**inputs) -> np.ndarray:
    enc = np.ascontiguousarray(
        np.asarray(inputs["encoder_final_hidden_layers"], dtype=np.float32)
    )
    q_all, ut_host = _host_prep(
        inputs["previous_decoder_hidden_state"],
        inputs["W_w"],
        inputs["W_b"],
        inputs["U_w"],
        inputs["U_b"],
        inputs["v"],
    )

    nc = _get_nc()
    in_maps = []
    for i in range(NCORES):
        sl = slice(i * BPC, (i + 1) * BPC)
        in_maps.append(
            {
                "enc": enc[sl],
                "qrow": np.ascontiguousarray(q_all[sl].reshape(1, BPC * H)),
                "ut": ut_host,
            }
        )
    try:
        res = run_bass_kernel_spmd(nc, in_maps, core_ids=list(range(NCORES)))
    except Exception:
        # a previously crashed run can leave a core wedged
        # (NRT_EXEC_UNIT_UNRECOVERABLE); one retry recovers
        res = run_bass_kernel_spmd(nc, in_maps, core_ids=list(range(NCORES)))
    return np.concatenate([r["out"] for r in res.results], axis=0)


if __name__ == "__main__":
    nc = build_bass()
    print("built ok")



# revision 17
# speedup vs baseline: 1.1993x; 1.1993x over previous
"""Bahdanau-attention kernel for Trainium2 (8 NeuronCores, data-parallel over batch).

Computes, for each batch b:
    q[b]    = v * (W_w @ prev[b] + W_b + U_b)            (host, tiny)
    U'      = v[:, None] * U_w                            (host, tiny)
    e[b,t]  = sum_h relu(q[b,h] + (U' @ enc[b,t])_h)      (device)
    alpha   = softmax(e[b, :])                            (device)
    out[b]  = sum_t alpha[t] * enc[b,t,:]                 (device)

The v>0 fold is exact: v_h * relu(x_h) == relu(v_h * x_h) for v_h >= 0.

Device strategy (per core: 4 batches, enc slice [4, 4096, 1024] fp32 = 64 MB
streamed from HBM exactly once, cast fp32->fp16 during the DMA — fp16's
10-bit mantissa matches the tf32-grade rounding f32r gives on HW, at half
the byte width, 1 cyc/row PE transposes, and FWL fast weight loads).
Throughput notes (HW-measured): enc arrives via 2 MB 4-tile SWDGE cast-DMAs
(512 KB DMAs are issue-bound at ~1.7 us; 2 MB reach >500 GB/s/core even with
all 8 cores streaming); the PSUM->SBUF evacuation runs as tensor_tensor(+0)
so the DVE stays in 1-port mode and never locks GpSimd/SWDGE out of the
shared SBUF port; batch emission is software-pipelined (batch b's softmax +
pass-2 are emitted after batch b+1's pass-1) with enc_pool sized for two
full batches so the PE stays fed across batch boundaries. Finer-grained
interleaves and pool rebalances all measured worse (see memory notes):
  - enc tiles [t=128, c=1024] fp16 stay SBUF-resident for the batch.
  - PE transposes each tile chunk-wise to [c, t] (fp16, PSUM), DVE copies
    the result to SBUF.
  - U-matmul in fp16 accumulates [t=128, h=256] in fp32 PSUM on top of a
    ones-row x q bias matmul.
  - ACT fused relu+row-reduce produces the energy column per tile.
  - Exact fp32 two-level softmax: per-partition max shift via the ACT bias,
    then a one-partition fixup; cross-partition gather/scatter rides PE
    transposes / a K=1 matmul.
  - Pass-2 weighted sum: alpha column as stationary, natural enc tile as
    moving operand, accumulated into PSUM [1, 1024].

Toolchain notes: the module is built as a Bacc (not raw Bass) so multi-wait
instructions get legalized into event semaphores and the walrus single-wait
LDWEIGHTS limit is respected. Matmul inputs must not mix 16/32-bit dtypes;
the softmax's per-partition max is rounded to fp16 FIRST and the rounded
value used in both exponents so z'*g composes exactly.
"""

import sys

import numpy as np

sys.path.insert(0, "/opt/trn_rl_repo")

import concourse.bacc as bacc
import concourse.mybir as mybir
import concourse.tile as tile
from concourse.bass import ts
from concourse.bass_utils import run_bass_kernel_spmd
from concourse.masks import make_identity

B, T, C, H, D = 32, 4096, 1024, 256, 512
NCORES = 8
BPC = B // NCORES  # batches per core

F32 = mybir.dt.float32
F32R = mybir.dt.float32r
F16 = mybir.dt.float16
BF16 = mybir.dt.bfloat16

P = 128            # partitions / t-tile size
CK = C // P        # 8 c-chunks per tile
NT = T // P        # 32 t-tiles per batch


def build_bass(bpc: int = BPC, n_tiles: int = NT, repeat: int = 1):
    nc = bacc.Bacc(target_bir_lowering=False, trn_type="TRN2")

    enc = nc.dram_tensor("enc", [bpc, n_tiles * P, C], F32, kind="ExternalInput")
    # q rows packed on one partition: [1, bpc*H]
    qrow = nc.dram_tensor("qrow", [1, bpc * H], F32, kind="ExternalInput")
    # U' transposed, pre-arranged host-side as [p, chunk, h] with c = chunk*128 + p
    ut = nc.dram_tensor("ut", [P, CK, H], F32, kind="ExternalInput")
    out = nc.dram_tensor("out", [bpc, C], F32, kind="ExternalOutput")

    enc_ap = enc.ap()
    out_ap = out.ap()

    with tile.TileContext(nc) as tc:
        TG = 4  # t-tiles per DMA (2 MB transfers amortize SWDGE issue cost)
        with (
            tc.tile_pool(name="singles", bufs=1) as singles,
            tc.tile_pool(name="enc_pool", bufs=2 * (n_tiles // TG)) as enc_pool,
            tc.tile_pool(name="encT_pool", bufs=3) as encT_pool,
            tc.tile_pool(name="relu_pool", bufs=3) as relu_pool,
            tc.tile_pool(name="batch_pool", bufs=3) as batch_pool,
            tc.tile_pool(name="small_pool", bufs=2) as small_pool,
            tc.tile_pool(name="outst_pool", bufs=2) as outst_pool,
            tc.tile_pool(name="ps_tp", bufs=3, space="PSUM") as ps_tp,
            tc.tile_pool(name="ps_um", bufs=3, space="PSUM") as ps_um,
            tc.tile_pool(name="ps_c", bufs=1, space="PSUM") as ps_c,
        ):
            # --- constants, all funneled through DVE so PE sees one clock ---
            ident_stage = singles.tile([P, P], F32)
            make_identity(nc, ident_stage)
            ut_stage = singles.tile([P, CK, H], F32)
            nc.gpsimd.dma_start(out=ut_stage, in_=ut.ap())
            q_stage = singles.tile([1, bpc * H], F32)
            nc.gpsimd.dma_start(out=q_stage, in_=qrow.ap())

            ones_row_f = singles.tile([1, P], F32)
            nc.vector.memset(ones_row_f, 1.0)
            ones_row = singles.tile([1, P], F16)
            nc.vector.tensor_copy(ones_row, ones_row_f)
            q_s = singles.tile([1, bpc * H], F16)
            nc.vector.tensor_copy(q_s, q_stage)
            ut_s = singles.tile([P, CK, H], F16)
            nc.vector.tensor_copy(ut_s, ut_stage)
            ident_h = singles.tile([P, P], F16)
            nc.vector.tensor_copy(ident_h, ident_stage)
            # zeros operand: lets the PSUM->SBUF evacuation run as
            # tensor_tensor(+0), which only uses DVE's dedicated 1-port mode.
            # A plain tensor_copy enters 2-port perf mode and locks GpSimd out
            # of the shared SBUF port pair, starving SWDGE descriptor
            # generation for the enc cast-DMAs (documented ~5x DMA stall).
            zeros_c = singles.tile([P, C], F16)
            nc.vector.memset(zeros_c, 0.0)

            def pass1(b):
                enc_tiles = []
                e_buf = batch_pool.tile([P, n_tiles], F32, tag="ebuf")
                for jg in range(n_tiles // TG):
                    enc_g = enc_pool.tile([P, TG, C], F16, tag="enc")
                    src_ap = enc_ap[b, ts(jg, TG * P), :].rearrange(
                        "(a p) c -> p a c", p=P
                    )
                    nc.gpsimd.dma_start(out=enc_g, in_=src_ap)
                    enc_tiles.extend(enc_g[:, tt, :] for tt in range(TG))
                for j in range(n_tiles):
                    enc_t = enc_tiles[j]

                    # transpose per half: 4 chunks [t,c]->[c,t] into one
                    # PSUM bank, then one DVE copy [128, 512] to SBUF
                    encT = encT_pool.tile([P, C], F16, tag="encT")
                    tp = ps_tp.tile([P, C], F16, tag="tp")
                    for k in range(CK):
                        nc.tensor.transpose(
                            tp[:, ts(k, P)], enc_t[:, ts(k, P)], ident_h
                        )
                    nc.vector.tensor_add(encT, tp, zeros_c)

                    # U-matmul: psum[t, h] = q[h] + sum_c encT[c,t]^T ut[c,h]
                    um = ps_um.tile([P, H], F32, tag="um")
                    nc.tensor.matmul(
                        um,
                        ones_row,
                        q_s[:, b * H : (b + 1) * H],
                        start=True,
                        stop=False,
                    )
                    for k in range(CK):
                        nc.tensor.matmul(
                            um,
                            encT[:, ts(k, P)],
                            ut_s[:, k, :],
                            start=False,
                            stop=(k == CK - 1),
                        )

                    # e[t] = sum_h relu(um[t, h])  (ACT, fused reduce)
                    relu_sc = relu_pool.tile([P, H], BF16, tag="relu")
                    nc.scalar.activation(
                        out=relu_sc,
                        in_=um,
                        func=mybir.ActivationFunctionType.Relu,
                        accum_out=e_buf[:, j : j + 1],
                    )
                return enc_tiles, e_buf

            def tail(b, enc_tiles, e_buf):
                # ------- softmax (exact fp32, two-level, PE transposes) -------
                # z'[p,j] = exp(e[p,j] - mp[p]) with the per-partition max mp
                # (ACT bias is per-partition, so no broadcast needed), then a
                # one-partition fixup computes g[p] = exp(mp[p]-M)/S and
                # alpha = z' * g  ==  exp(e-M)/S exactly. Cross-partition
                # gather/scatter rides the PE transpose (sub-us) instead of
                # SBUF->SBUF DMA (~1.5us fixed each).
                ms = small_pool.tile([P, 2], F32, tag="ms")
                nc.vector.tensor_reduce(
                    ms[:, 0:1], e_buf, axis=mybir.AxisListType.X,
                    op=mybir.AluOpType.max,
                )
                ms_r = small_pool.tile([P, 2], F16, tag="ms_r")
                nc.vector.tensor_copy(ms_r[:, 0:1], ms[:, 0:1])
                mpneg = small_pool.tile([P, 1], F32, tag="mpneg")
                nc.vector.tensor_scalar_mul(mpneg, ms_r[:, 0:1], -1.0)
                z = batch_pool.tile([P, n_tiles], F32, tag="z")
                nc.scalar.activation(
                    out=z,
                    in_=e_buf,
                    func=mybir.ActivationFunctionType.Exp,
                    bias=mpneg,
                    accum_out=ms[:, 1:2],
                )
                # gather each column onto partition 0 via PE transposes
                # (f32r rounding copies keep the BIR verifier happy)
                nc.vector.tensor_copy(ms_r[:, 1:2], ms[:, 1:2])
                mrow_ps = ps_tp.tile([1, P], F16, tag="tp")
                nc.tensor.transpose(mrow_ps, ms_r[:, 0:1], ident_h)
                srow_ps = ps_tp.tile([1, P], F16, tag="tp")
                nc.tensor.transpose(srow_ps, ms_r[:, 1:2], ident_h)
                mrow = small_pool.tile([1, P], F32, tag="mrow")
                nc.vector.tensor_copy(mrow, mrow_ps)
                srow = small_pool.tile([1, P], F32, tag="srow")
                nc.vector.tensor_copy(srow, srow_ps)
                mtot = small_pool.tile([1, 1], F32, tag="mtot")
                nc.vector.tensor_reduce(
                    mtot, mrow, axis=mybir.AxisListType.X, op=mybir.AluOpType.max
                )
                mtneg = small_pool.tile([1, 1], F32, tag="mtneg")
                nc.vector.tensor_scalar_mul(mtneg, mtot, -1.0)
                grow = small_pool.tile([1, P], F32, tag="grow")
                nc.scalar.activation(
                    out=grow,
                    in_=mrow,
                    func=mybir.ActivationFunctionType.Exp,
                    bias=mtneg,
                )
                wrow = small_pool.tile([1, P], F32, tag="wrow")
                nc.vector.tensor_mul(wrow, grow, srow)
                stot = small_pool.tile([1, 1], F32, tag="stot")
                nc.vector.tensor_reduce(
                    stot, wrow, axis=mybir.AxisListType.X, op=mybir.AluOpType.add
                )
                rec = small_pool.tile([1, 1], F32, tag="rec")
                nc.vector.reciprocal(rec, stot)
                gsrow = small_pool.tile([1, P], F32, tag="gsrow")
                nc.vector.tensor_scalar_mul(gsrow, grow, rec)
                gsrow_r = small_pool.tile([1, P], F16, tag="gsrow_r")
                nc.vector.tensor_copy(gsrow_r, gsrow)
                # scatter g[p]/S back to one element per partition via a
                # K=1 matmul: out[p, 0] = gsrow[p] * 1
                gscol_ps = ps_tp.tile([P, 32], F32, tag="tp")
                nc.tensor.matmul(
                    gscol_ps, gsrow_r, ones_row[:, 0:32], start=True, stop=True
                )
                gscol = small_pool.tile([P, 1], F32, tag="gscol")
                nc.vector.tensor_copy(gscol, gscol_ps[:, 0:1])
                alpha = batch_pool.tile([P, n_tiles], F16, tag="alpha")
                nc.vector.tensor_scalar_mul(alpha, z, gscol)

                # ---------------- pass 2: weighted sum ----------------
                cps = ps_c.tile([1, 2, D], F32, tag="cps")
                for j in range(n_tiles):
                    for h in range(2):
                        nc.tensor.matmul(
                            cps[:, h, :],
                            alpha[:, j : j + 1],
                            enc_tiles[j][:, ts(h, D)],
                            start=(j == 0),
                            stop=(j == n_tiles - 1),
                        )
                c_st = outst_pool.tile([1, C], F32, tag="cst")
                nc.vector.tensor_copy(c_st, cps.rearrange("p a b -> p (a b)"))
                nc.sync.dma_start(out=out_ap[b : b + 1, :], in_=c_st)

            def batches():
                # software-pipelined emission: batch b's softmax+pass-2 is
                # emitted AFTER batch b+1's pass-1, so the scheduler keeps the
                # PE fed with b+1 transposes/U-matmuls while b's softmax
                # dependency chain runs on DVE/ACT. enc_pool holds exactly two
                # batches (16 groups).
                prev = None
                for b in range(bpc):
                    cur = (b, *pass1(b))
                    if prev is not None:
                        tail(*prev)
                    prev = cur
                tail(*prev)

            if repeat == 1:
                batches()
            else:
                with tc.For_i(0, repeat, 1):
                    batches()

    return nc


_NC_CACHE: dict = {}


def _get_nc(bpc=BPC, n_tiles=NT):
    key = (bpc, n_tiles)
    if key not in _NC_CACHE:
        nc = build_bass(bpc, n_tiles)
        if not nc.is_finalized():
            nc.finalize()
        _NC_CACHE[key] = nc
    return _NC_CACHE[key]


def _host_prep(previous_decoder_hidden_state, W_w, W_b, U_w, U_b, v):
    prev = np.asarray(previous_decoder_hidden_state, dtype=np.float32)[:, 0, :]
    W_w = np.asarray(W_w, dtype=np.float32)
    U_w = np.asarray(U_w, dtype=np.float32)
    v = np.asarray(v, dtype=np.float32)
    bias = np.asarray(W_b, dtype=np.float32) + np.asarray(U_b, dtype=np.float32)
    q_all = (v[None, :] * (prev @ W_w.T + bias)).astype(np.float32)  # [B, H]
    up = (v[:, None] * U_w).astype(np.float32)  # [H, C]
    # ut_host[p, k, h] = up.T[k*128 + p, h]
    ut_host = np.ascontiguousarray(up.T.reshape(CK, P, H).transpose(1, 0, 2))
    return q_all, ut_host


def kernel(**inputs) -> np.ndarray:
    enc = np.ascontiguousarray(
        np.asarray(inputs["encoder_final_hidden_layers"], dtype=np.float32)
    )
    q_all, ut_host = _host_prep(
        inputs["previous_decoder_hidden_state"],
        inputs["W_w"],
        inputs["W_b"],
        inputs["U_w"],
        inputs["U_b"],
        inputs["v"],
    )

    nc = _get_nc()
    in_maps = []
    for i in range(NCORES):
        sl = slice(i * BPC, (i + 1) * BPC)
        in_maps.append(
            {
                "enc": enc[sl],
                "qrow": np.ascontiguousarray(q_all[sl].reshape(1, BPC * H)),
                "ut": ut_host,
            }
        )
    try:
        res = run_bass_kernel_spmd(nc, in_maps, core_ids=list(range(NCORES)))
    except Exception:
        # a previously crashed run can leave a core wedged
        # (NRT_EXEC_UNIT_UNRECOVERABLE); one retry recovers
        res = run_bass_kernel_spmd(nc, in_maps, core_ids=list(range(NCORES)))
    return np.concatenate([r["out"] for r in res.results], axis=0)


if __name__ == "__main__":
    nc = build_bass()
    print("built ok")

